# revision 1
# baseline (speedup 1.0000x reference)
"""MoE (top-2 of 8 experts, d=1024, h=4096) on 8 Trainium2 NeuronCores.

Strategy (expert-parallel, per sharding hint):
  - Host: gating (fp64 logits/softmax/top-2 — tie margins on this problem are
    ~1e-5, far above fp32 rounding noise, so host routing matches the
    reference's fp32 top-k), per-expert token gather, pad to capacity C.
  - Device (core e = expert e): hidT = relu(W1_e.T @ x_e.T + b1_e) then
    ye = hidT.T @ W2_e, both as K-tiled 128x128x512 matmuls in float32r
    (full PE rate, ~1e-4 matmul rel err).
  - Host: out[tok_e] += gate_e * (ye + b2_e)  (scatter-combine).

Self-contained: hardcodes all shapes; only imports concourse (system lib).
"""

import os

os.environ.setdefault("JAX_PLATFORMS", "")

import numpy as np

import concourse.bacc as bacc
import concourse.mybir as mybir
import concourse.tile as tile
from concourse.bass_utils import run_bass_kernel_spmd

P = 128
D = 1024  # embed dim
H = 4096  # hidden dim
E = 8  # experts
TOPK = 2
KD = D // P  # 8  k-tiles over embed
KH = H // P  # 32 k-tiles over hidden
NCORES = 8
FD = 512  # matmul moving free dim (one PSUM bank of fp32)

_compiled = {}
LAST_RESULT = None  # BassKernelResults of the most recent run (for test harness)


def _phase1(nc, tc, rs, C, chunks, xt_d, w1_d, b1_d, hid_cs):
    f32 = mybir.dt.float32
    f32r = mybir.dt.float32r
    relu = mybir.ActivationFunctionType.Relu
    TN = len(chunks)
    with (
        tc.tile_pool(name=rs + "xts_p", bufs=1) as xts_p,
        tc.tile_pool(name=rs + "b1_p", bufs=1) as b1_p,
        tc.tile_pool(name=rs + "w1_p", bufs=int(os.environ.get("MOE_W1B", "4"))) as w1_p,
        tc.tile_pool(name=rs + "hb_p", bufs=1) as hb_p,
        tc.tile_pool(name=rs + "ps1", bufs=int(os.environ.get("MOE_PS1", "4")), space="PSUM") as ps1,
    ):
        def load_w1(hm):
            w1t = w1_p.tile([P, KD, P], f32r, tag="w1t", name=rs + f"w1t_{hm}")
            nc.sync.dma_start(w1t[:], w1_d[:, hm])
            return w1t

        # Issue order matters: HWDGE dispatches in program order, so the
        # first matmul group's deps (w1t[0] + x chunk 0) are issued first.
        w1_pre = {0: load_w1(0)}
        # x chunks as separate per-k tiles so the first matmul group only
        # waits on its own 8 pieces (~2MB), not the whole 10MB load.
        xc = [[None] * KD for _ in range(TN)]
        for tn, (off, w) in enumerate(chunks):
            for k0 in range(0, KD, 2):
                t = xts_p.tile(
                    [P, 2, w], f32r, tag=f"x_{tn}_{k0}", name=rs + f"x_{tn}_{k0}"
                )
                nc.sync.dma_start(t[:], xt_d[:, k0 : k0 + 2, off : off + w])
                xc[tn][k0] = t[:, 0, :]
                xc[tn][k0 + 1] = t[:, 1, :]
            if tn == 0:
                # b1 is first needed at the first eviction, not the first
                # matmul: issue it after chunk 0's loads.
                b1s = b1_p.tile([P, KH], f32, name=rs + "b1s")
                nc.sync.dma_start(b1s[:], b1_d[:])
            if tn < 3:  # prefetch next stationary tiles early
                w1_pre[tn + 1] = load_w1(tn + 1)
        # PE emission order: the first W hm rows are swept tn-major (wave
        # order) so the earliest matmuls only touch x chunks that have
        # already landed; the rest are hm-major. Each (hm, tn) psum group is
        # independent, so this only reorders work.
        W = int(os.environ.get("MOE_W", "2")) if TN > 1 else 0
        sched = [(hm, tn) for tn in range(TN) for hm in range(W)]
        sched += [(hm, tn) for hm in range(W, KH) for tn in range(TN)]

        w1ts, done = {}, {}
        KQ1 = KH // 4
        for hm, tn in sched:
            if hm not in w1ts:
                w1ts[hm] = w1_pre.pop(hm) if hm in w1_pre else load_w1(hm)
                done[hm] = 0
            off, w = chunks[tn]
            pt = ps1.tile([P, FD], f32, tag="ps1", name=rs + f"ps1_{hm}_{tn}")
            for k in range(KD):
                nc.tensor.matmul(
                    pt[:, :w],
                    w1ts[hm][:, k, :],
                    xc[tn][k],
                    start=(k == 0),
                    stop=(k == KD - 1),
                )
            # evict through a small per-chunk staging tile (ACT does
            # relu+bias, then the hid write DMAs it straight out on the ACT
            # HWDGE ring so phase-2 loads (SP ring) aren't queued behind it)
            hbst = int(os.environ.get("MOE_HBST", "12")) if C <= 2560 else 6
            hb = hb_p.tile([P, w], f32r, tag="hbst", bufs=hbst, name=rs + f"hb_{hm}_{tn}")
            nc.scalar.activation(
                hb[:, :w], pt[:, :w], relu, bias=b1s[:, hm : hm + 1]
            )
            nc.scalar.dma_start(
                hid_cs[tn][hm // KQ1][:, :, hm % KQ1, :].transpose([1, 0, 2]),
                hb.rearrange("p (t q) -> p t q", q=P),
            )
            done[hm] += 1
            if done[hm] == TN:
                del w1ts[hm]  # release references; pool slots recycle


W2HEAD = 8  # w2 chunks living in the persistent pool (loadable during phase 1)


def _phase2(nc, tc, rs, C, chunks, w2_d, hid_cs, ye_d, hd_p, w2h_p, ps2):
    f32 = mybir.dt.float32
    f32r = mybir.dt.float32r
    TM = C // P
    with (
        tc.tile_pool(name=rs + "w2_p", bufs=1) as w2_p,
        tc.tile_pool(name=rs + "out_p", bufs=int(os.environ.get("MOE_OUTB", "3"))) as out_p,
    ):

        HDS = 4  # hd k-split (must match the 4-way hid_cs DRAM split)
        KQ = KH // HDS

        def load_hd(tm):
            cidx = next(
                i for i, (off, w) in enumerate(chunks) if off // P <= tm < (off + w) // P
            )
            local = tm - chunks[cidx][0] // P
            parts = []
            for q in range(HDS):
                hdq = hd_p.tile(
                    [P, KQ, P], f32r, tag=f"hd{q}", name=rs + f"hd_{tm}_{q}"
                )
                nc.sync.dma_start(hdq[:], hid_cs[cidx][q][local])
                parts.append(hdq)
            return parts

        # Issue order: w2 head + first token tile's data before the bulk w2
        # load, so the first phase-2 matmul isn't queued behind 16MB of w2 on
        # the in-order HWDGE ring. Head w2 + hd live in pools hoisted outside
        # phase 1's, so these loads can run during phase 1's tail.
        w2ts = []
        for k in range(W2HEAD):
            w2t = w2h_p.tile([P, D], f32r, tag=f"w2_{k}", name=rs + f"w2_{k}")
            nc.sync.dma_start(w2t[:], w2_d[k])
            w2ts.append(w2t)
        hd_pre = {0: load_hd(0)}
        for k in range(W2HEAD, KH):
            w2t = w2_p.tile([P, D], f32r, tag=f"w2_{k}", name=rs + f"w2_{k}")
            nc.sync.dma_start(w2t[:], w2_d[k])
            w2ts.append(w2t)
            if k == 15:
                hd_pre[1] = load_hd(1)
        hd_pre[2] = load_hd(2)
        for tm in range(TM):
            hd = hd_pre.pop(tm) if tm in hd_pre else load_hd(tm)
            ob = out_p.tile([P, D], f32, tag="ob", name=rs + f"ob_{tm}")
            for n in range(D // FD):
                pt2 = ps2.tile([P, FD], f32, tag="ps2", name=rs + f"ps2_{tm}_{n}")
                for k in range(KH):
                    nc.tensor.matmul(
                        pt2[:],
                        hd[k // KQ][:, k % KQ, :],
                        w2ts[k][:, n * FD : (n + 1) * FD],
                        start=(k == 0),
                        stop=(k == KH - 1),
                    )
                nc.vector.tensor_copy(ob[:, n * FD : (n + 1) * FD], pt2[:])
            nc.scalar.dma_start(ye_d[tm], ob[:])


def _build(C, reps=1):
    """Per-core SPMD program for capacity-C tokens through one expert.

    reps>1 repeats the whole program back-to-back (timing experiments only).
    """
    if (C, reps) in _compiled:
        return _compiled[(C, reps)]
    f32 = mybir.dt.float32
    f32r = mybir.dt.float32r
    TM = C // P  # token tiles (GEMM2 stationary / output rows)
    # GEMM1 moving chunks: 512s plus one remainder (multiple of 128; N>=256
    # keeps fp32r at full rate, a 128 tail is negligible)
    chunks = []
    off = 0
    CW = int(os.environ.get("MOE_CW", "0"))
    if CW and C % CW == 0:  # uniform chunk-width experiment knob
        while off < C:
            chunks.append((off, CW))
            off += CW
    else:
        if C >= 768:  # small first chunk -> first matmul group starts sooner
            chunks.append((0, 256))
            off = 256
        while off < C:
            w = min(FD, C - off)
            chunks.append((off, w))
            off += w

    nc = bacc.Bacc(None, target_bir_lowering=False)
    # xt host layout [P, KD, C]: xt[p, k, c] = x[tok_c, k*128+p] (transposed)
    xt_d = nc.dram_tensor("xt", [P, KD, C], f32r, kind="ExternalInput")
    # w1 host layout [P, KH, KD, P]: w1[p, hm, k, j] = W1[k*128+p, hm*128+j]
    # -> per-hm stationary-tile loads are contiguous 4KB per partition.
    w1_d = nc.dram_tensor("w1", [P, KH, KD, P], f32r, kind="ExternalInput")
    b1_d = nc.dram_tensor("b1", [P, KH], f32, kind="ExternalInput")
    w2_d = nc.dram_tensor("w2", [KH, P, D], f32r, kind="ExternalInput")
    ye_d = nc.dram_tensor("ye", [TM, P, D], f32, kind="ExternalOutput")

    with tile.TileContext(nc) as tc:
        with tc.tile_pool(name="dram", bufs=1, space="DRAM") as dram:
            # hidT blocks: [token-tile, hidden-in-tile (partition), hm,
            # token-in-tile] -> phase-2 reads are contiguous 16KB/partition.
            # One DRAM tile per token chunk so phase-2's first loads only
            # depend on writes to their own chunk.
            # ... and per k-quarter, so phase-2's early hd quarters depend
            # only on the phase-1 rows that produced them (DRAM deps are
            # whole-tile).
            hid_cs = [
                [
                    dram.tile(
                        [w // P, P, KH // 4, P],
                        f32r,
                        tag=f"hidc_{i}_{q}",
                        name=f"hidc_{i}_{q}",
                    )
                    for q in range(4)
                ]
                for i, (off, w) in enumerate(chunks)
            ]
            for rep in range(reps):
                rs = "" if rep == 0 else f"r{rep}_"
                # hd/w2-head/psum2 pools are hoisted outside phase 1's pools
                # so phase 2's first loads don't wait for phase-1 SBUF release.
                with (
                    tc.tile_pool(name=rs + "hd_p", bufs=3) as hd_p,
                    tc.tile_pool(name=rs + "w2h_p", bufs=1) as w2h_p,
                    tc.tile_pool(name=rs + "ps2", bufs=int(os.environ.get("MOE_PS2", "4")), space="PSUM") as ps2,
                ):
                    _phase1(nc, tc, rs, C, chunks, xt_d, w1_d, b1_d, hid_cs)
                    _phase2(
                        nc, tc, rs, C, chunks, w2_d, hid_cs, ye_d, hd_p, w2h_p, ps2
                    )

    nc.compile()
    _compiled[(C, reps)] = nc
    return nc


def kernel(x, Wg, bg, W1, b1, W2, b2):
    global LAST_RESULT
    x = np.ascontiguousarray(x, dtype=np.float32)
    B, S, d = x.shape
    assert d == D
    T = B * S
    xf = x.reshape(T, d)

    # ---- Host gating/routing (fp64) ----
    logits = xf.astype(np.float64) @ Wg.astype(np.float64) + bg.astype(np.float64)
    mx = logits.max(axis=1, keepdims=True)
    ex = np.exp(logits - mx)
    probs = ex / ex.sum(axis=1, keepdims=True)
    order = np.argsort(-logits, axis=1, kind="stable")  # ties -> lower index
    top = order[:, :TOPK]  # [T, 2]
    gsel = np.take_along_axis(probs, top, axis=1).astype(np.float32)

    toks, gates = [], []
    for e in range(E):
        pos = top == e  # [T, 2]
        sel = pos.any(axis=1)
        toks.append(np.nonzero(sel)[0])
        gates.append((gsel * pos).sum(axis=1)[sel].astype(np.float32))

    maxcnt = max(len(t) for t in toks)
    # SBUF budget caps resident x at 4096 tokens/core; batch if routing is
    # ever concentrated enough to exceed that (never for balanced gating).
    MAXC = 2944
    nb = max(1, -(-maxcnt // MAXC))
    C = max(P, ((-(-maxcnt // nb) + P - 1) // P) * P)

    w_maps = []  # per-expert weight shards (batch-invariant)
    for e in range(E):
        w_maps.append(
            {
                "w1": np.ascontiguousarray(
                    np.asarray(W1[e], dtype=np.float32)
                    .reshape(KD, P, KH, P)
                    .transpose(1, 2, 0, 3)
                ),
                "b1": np.ascontiguousarray(
                    np.asarray(b1[e], dtype=np.float32).reshape(KH, P).T
                ),
                "w2": np.ascontiguousarray(W2[e], dtype=np.float32).reshape(KH, P, D),
            }
        )

    nc = _build(C)
    out = np.zeros((T, D), np.float32)
    b2f = np.asarray(b2, dtype=np.float32)
    for b in range(nb):
        in_maps = []
        btoks = []
        for e in range(E):
            tk = toks[e][b * C : (b + 1) * C]
            btoks.append(tk)
            xe = np.zeros((C, D), np.float32)
            xe[: len(tk)] = xf[tk]
            in_maps.append(
                {
                    "xt": np.ascontiguousarray(
                        xe.T.reshape(KD, P, C).transpose(1, 0, 2)
                    ),
                    **w_maps[e],
                }
            )
        res = run_bass_kernel_spmd(nc, in_maps, core_ids=list(range(NCORES)))
        LAST_RESULT = res
        for e in range(E):
            cnt = len(btoks[e])
            if cnt == 0:
                continue
            ye = res.results[e]["ye"].reshape(C, D)[:cnt]
            g = gates[e][b * C : b * C + cnt]
            out[btoks[e]] += g[:, None] * (ye + b2f[e])
    return out.reshape(B, S, D)



# revision 2
# speedup vs baseline: 1.1340x; 1.1340x over previous
"""MoE (top-2 of 8 experts, d=1024, h=4096) on 8 Trainium2 NeuronCores.

Expert-parallel with 2-slot load balancing: each core runs two weight
"segments" (C1 + C2 tokens, same shapes on every core = SPMD); the host
packs expert token sets into the 16 slots (an expert may span several
slots) so per-core work is C1+C2 ~ 2176 instead of max-expert-count 2304.

Per segment, fused on-chip FFN in bf16:
  G1: hidT[h, tok] = relu(W1.T @ x.T + b1)  -- psum f32, ACT evicts to bf16
  G2: ye[tok, d]   = hidT.T @ W2            -- hid stays in SBUF (no DRAM
                                               round-trip), W2 resident bf16
Emission keeps the PE stream gap-free (the TimelineSim p-state model
penalizes any idle with ~3us of half-rate ramp); a short warmup matmul
chain covers the initial DMA wait.

Host: fp64 gating/top-2 routing, slot packing, bf16 conversion, and the
gate-weighted combine (+b2).

Self-contained: hardcodes all shapes; only imports concourse (system lib).
"""

import os

os.environ.setdefault("JAX_PLATFORMS", "")

import numpy as np

import concourse.bacc as bacc
import concourse.mybir as mybir
import concourse.tile as tile
from concourse.bass_utils import run_bass_kernel_spmd

P = 128
D = 1024  # embed dim
H = 4096  # hidden dim
E = 8  # experts
TOPK = 2
KD = D // P  # 8  k-tiles over embed
KH = H // P  # 32 k-tiles over hidden
NCORES = 8
FD = 512  # psum bank free dim (f32)

NWARM = int(os.environ.get("MOE_NWARM", "18"))
W1BUFS = int(os.environ.get("MOE_W1B", "6"))
W1PRE = int(os.environ.get("MOE_W1PRE", "4"))

_compiled = {}
LAST_RESULT = None


def _chunks(C, first):
    """Token chunks for G1's moving dim: optional 384-token first chunk (small
    x0 DMA for a fast start, big enough that the first wave of hm groups
    covers the x1/x2 transfer time), then 512s, remainder >=128."""
    out = []
    off = 0
    if first and C >= 1024:
        out.append((0, 384))
        off = 384
    while off < C:
        w = min(FD, C - off)
        if C - off - w == 128:  # avoid a 128 tail; split 384+256 etc.
            w -= 128
        out.append((off, w))
        off += w
    return out


def _build(C1, C2):
    if (C1, C2) in _compiled:
        return _compiled[(C1, C2)]
    f32 = mybir.dt.float32
    bf16 = mybir.dt.bfloat16
    relu = mybir.ActivationFunctionType.Relu

    nc = bacc.Bacc(None, target_bir_lowering=False)
    segs = []
    for s, C in ((0, C1), (1, C2)):
        xt_d = nc.dram_tensor(f"xt{s}", [P, KD, C], bf16, kind="ExternalInput")
        w1_d = nc.dram_tensor(f"w1s{s}", [P, KH, KD, P], bf16, kind="ExternalInput")
        b1_d = nc.dram_tensor(f"b1s{s}", [P, KH], f32, kind="ExternalInput")
        w2_d = nc.dram_tensor(f"w2s{s}", [KH, P, D], bf16, kind="ExternalInput")
        # ye in bf16: halves the output DMA (incl. the one on the critical
        # tail) and enables the DVE 2x copy mode; ~0.1% extra output error.
        ye_d = nc.dram_tensor(f"ye{s}", [C // P, P, D], bf16, kind="ExternalOutput")
        segs.append((s, C, xt_d, w1_d, b1_d, w2_d, ye_d))

    with tile.TileContext(nc) as tc:
        with (
            tc.tile_pool(name="warm_p", bufs=1) as warm_p,
            tc.tile_pool(name="x_p", bufs=1) as x_p,
            tc.tile_pool(name="b1_p", bufs=1) as b1_p,
            tc.tile_pool(name="w1_p", bufs=W1BUFS) as w1_p,
            tc.tile_pool(name="hid_p", bufs=1) as hid_p,
            tc.tile_pool(name="w2_p", bufs=1) as w2_p,
            tc.tile_pool(name="ob_p", bufs=3) as ob_p,
            tc.tile_pool(name="ps1", bufs=4, space="PSUM") as ps1,
            tc.tile_pool(name="ps2", bufs=4, space="PSUM") as ps2,
        ):
            # --- PE warmup: keep the tensor engine busy (p-state ramp) while
            # the first real weight/x DMAs land.
            if NWARM:
                warm = warm_p.tile([P, 2 * P], bf16, name="warm")
                nc.vector.memset(warm[:], 0.125)
                for i in range(NWARM):
                    wp = ps1.tile([P, FD], f32, tag="ps1", name=f"wps_{i}")
                    nc.tensor.matmul(
                        wp[:, :P], warm[:, :P], warm[:, P:], start=True, stop=True
                    )

            w2ts = {}
            st = [
                {"w1t": {}, "xc": {}, "chunks": _chunks(seg[1], first=(seg[0] == 0))}
                for seg in segs
            ]

            def load_w1(s, hm, eng):
                t = w1_p.tile([P, KD, P], bf16, tag="w1", name=f"w1_{s}_{hm}")
                eng.dma_start(t[:], segs[s][3][:, hm])
                st[s]["w1t"][hm] = t

            def load_x(s, ci, eng):
                # x and b1 tags are SHARED across segments (rotating bufs):
                # seg1's loads then WAR on seg0's last reads, so they cannot
                # dispatch at t=0 and front-run seg0's critical startup
                # transfers on the shared DMA engines.
                off, w = st[s]["chunks"][ci]
                t = x_p.tile([P, KD, w], bf16, tag=f"x{ci}", name=f"x_{s}_{ci}")
                eng.dma_start(t[:], segs[s][2][:, :, off : off + w])
                st[s]["xc"][ci] = t

            def startup_loads(s, eng):
                # Ring order == PE consumption order: w1[0], x0, b1 (needed at
                # the FIRST eviction - psum recycling stalls on it), the rest
                # of the first stationary block, then x1, x2.
                load_x(s, 0, eng)
                load_w1(s, 0, eng)
                b1s = b1_p.tile([P, KH], f32, tag="b1", name=f"b1_{s}")
                eng.dma_start(b1s[:], segs[s][4][:])
                st[s]["b1"] = b1s
                for hm in range(1, W1BUFS):
                    load_w1(s, hm, eng)
                for ci in range(1, len(st[s]["chunks"])):
                    load_x(s, ci, eng)

            def g1(s):
                chunks = st[s]["chunks"]
                w1t, xc, b1s = st[s]["w1t"], st[s]["xc"], st[s]["b1"]
                C = segs[s][1]
                # Schedule: hm-major/chunk-inner (W1 streamed through W1BUFS
                # tiles), except seg0's first block runs chunk-0-only first (a
                # wave): early PE work needs only the small x0 while x1/x2 are
                # still in flight.
                NCH = len(chunks)
                sched = []
                for b0 in range(0, KH, W1BUFS):
                    hs = list(range(b0, min(b0 + W1BUFS, KH)))
                    if b0 == 0 and s == 0:
                        sched += [(hm, 0) for hm in hs]
                        sched += [(hm, c) for hm in hs for c in range(1, NCH)]
                    else:
                        sched += [(hm, c) for hm in hs for c in range(NCH)]

                # W2 tiles stream 2 per first-touched hm from hm index 8 on
                # (ACT ring): late enough to keep early DMA for x/W1, early
                # enough to finish well before G2 needs them. For s=1 each
                # load WARs on seg0 G2's last read of that k tile.
                hid = hid_p.tile([P, KH, C], bf16, tag="hid", name=f"hid_{s}")
                st[s]["hid"] = hid
                seen = set()
                w2_next = 0
                for hm, ci in sched:
                    if hm not in seen:
                        seen.add(hm)
                        nxt = hm + W1BUFS
                        if nxt < KH:
                            load_w1(s, nxt, nc.sync)
                        if len(seen) > 8:
                            for _ in range(2):
                                if w2_next < KH:
                                    t = w2_p.tile(
                                        [P, D], bf16, tag=f"w2_{w2_next}",
                                        name=f"w2_{s}_{w2_next}",
                                    )
                                    nc.scalar.dma_start(t[:], segs[s][5][w2_next])
                                    w2ts[w2_next] = t
                                    w2_next += 1
                    off, w = chunks[ci]
                    wt = w1t[hm]
                    pt = ps1.tile([P, FD], f32, tag="ps1", name=f"p1_{s}_{hm}_{ci}")
                    for k in range(KD):
                        nc.tensor.matmul(
                            pt[:, :w],
                            wt[:, k, :],
                            xc[ci][:, k, :],
                            start=(k == 0),
                            stop=(k == KD - 1),
                        )
                    nc.scalar.activation(
                        hid[:, hm, off : off + w],
                        pt[:, :w],
                        relu,
                        bias=b1s[:, hm : hm + 1],
                    )

            def g2(s):
                C, ye_d = segs[s][1], segs[s][6]
                hid = st[s]["hid"]
                TM = C // P
                for tm in range(TM):
                    ob = ob_p.tile([P, D], bf16, tag="ob", name=f"ob_{s}_{tm}")
                    for n in range(D // FD):
                        pt2 = ps2.tile([P, FD], f32, tag="ps2", name=f"p2_{s}_{tm}_{n}")
                        for k in range(KH):
                            nc.tensor.matmul(
                                pt2[:],
                                hid[:, k, tm * P : (tm + 1) * P],
                                w2ts[k][:, n * FD : (n + 1) * FD],
                                start=(k == 0),
                                stop=(k == KH - 1),
                            )
                        nc.vector.tensor_copy(ob[:, n * FD : (n + 1) * FD], pt2[:])
                        # ye rides the ACT ring, except the very last transfer
                        # which takes the (empty) SP ring so the final two
                        # output DMAs overlap instead of serializing.
                        last = s == 1 and tm == TM - 1 and n == 1
                        (nc.sync if last else nc.scalar).dma_start(
                            ye_d[tm][:, n * FD : (n + 1) * FD],
                            ob[:, n * FD : (n + 1) * FD],
                        )

            # Emission order: seg0 startup (ACT ring, exact FIFO control),
            # seg0 G1, then seg1 startup on the SP ring -- emitted HERE so its
            # queue position is behind seg0's WAR-gated w1 stream (otherwise
            # its dependency-free x loads would front-run seg0's critical
            # startup transfers on the shared DMA engines).
            startup_loads(0, nc.scalar)
            g1(0)
            startup_loads(1, nc.sync)
            g2(0)
            g1(1)
            g2(1)

    nc.compile()
    _compiled[(C1, C2)] = nc
    return nc


# ---------------- host side ----------------


def _try_pack(counts, C1, C2):
    """Assign each expert (a,b) = (#C1-slots, #C2-slots) covering its count,
    within 8 slots of each size. DFS biggest-expert-first, min-waste combos."""
    order = sorted(range(len(counts)), key=lambda e: -counts[e])
    assign = {}

    def dfs(i, n1, n2):
        if i == len(order):
            return True
        c = counts[order[i]]
        combos = [
            (a, b)
            for a in range(n1 + 1)
            for b in range(n2 + 1)
            if a * C1 + b * C2 >= c
        ]
        combos.sort(key=lambda ab: (ab[0] * C1 + ab[1] * C2, ab[0] + ab[1]))
        for a, b in combos[:6]:
            assign[order[i]] = (a, b)
            if dfs(i + 1, n1 - a, n2 - b):
                return True
        assign.pop(order[i], None)
        return False

    return dict(assign) if dfs(0, NCORES, NCORES) else None


def _solve_packing(counts):
    """Smallest T=C1+C2 (slot sizes, multiples of 128) with a feasible
    assignment; C1 capped so hid+W2+x fit in SBUF."""
    for T in range(2176, 4608 + 1, 128):
        for C1 in range((T + 255) // 256 * 128, min(T - 128, 1280) + 1, 128):
            C2 = T - C1
            if C2 < 128:
                continue
            a = _try_pack(counts, C1, C2)
            if a is not None:
                return C1, C2, a
    raise RuntimeError(f"no packing for counts={counts}")


def kernel(x, Wg, bg, W1, b1, W2, b2):
    global LAST_RESULT
    import ml_dtypes

    bf16 = ml_dtypes.bfloat16
    x = np.ascontiguousarray(x, dtype=np.float32)
    B, S, d = x.shape
    assert d == D
    T = B * S
    xf = x.reshape(T, d)

    # ---- Host gating/routing (fp64) ----
    logits = xf.astype(np.float64) @ Wg.astype(np.float64) + bg.astype(np.float64)
    mx = logits.max(axis=1, keepdims=True)
    ex = np.exp(logits - mx)
    probs = ex / ex.sum(axis=1, keepdims=True)
    top = np.argsort(-logits, axis=1, kind="stable")[:, :TOPK]  # ties -> lower idx
    gsel = np.take_along_axis(probs, top, axis=1).astype(np.float32)

    toks, gates = [], []
    for e in range(E):
        pos = top == e
        sel = pos.any(axis=1)
        toks.append(np.nonzero(sel)[0])
        gates.append((gsel * pos).sum(axis=1)[sel].astype(np.float32))
    counts = [len(t) for t in toks]

    C1, C2, assign = _solve_packing(counts)

    # Instantiate slots: per size, a list of (expert, tok_start, n_tokens).
    slots = {C1: [], C2: []}
    for e in range(E):
        a, b = assign[e]
        rem, off = counts[e], 0
        for cap, cnt in ((C1, a), (C2, b)):
            for _ in range(cnt):
                take = min(rem, cap)
                slots[cap].append((e, off, take))
                off += take
                rem -= take
    for cap in (C1, C2):
        while len(slots[cap]) < NCORES:
            slots[cap].append((0, 0, 0))  # unused slot: expert-0 weights, 0 toks

    # Per-expert device weight layouts (bf16), built once.
    w_maps = {}
    for e in set(s[0] for cap in (C1, C2) for s in slots[cap]):
        w_maps[e] = {
            "w1": np.ascontiguousarray(
                np.asarray(W1[e], dtype=np.float32)
                .reshape(KD, P, KH, P)
                .transpose(1, 2, 0, 3)
            ).astype(bf16),
            "b1": np.ascontiguousarray(
                np.asarray(b1[e], dtype=np.float32).reshape(KH, P).T
            ),
            "w2": np.ascontiguousarray(W2[e], dtype=np.float32)
            .reshape(KH, P, D)
            .astype(bf16),
        }

    nc = _build(C1, C2)
    in_maps = []
    for core in range(NCORES):
        m = {}
        for s, cap in ((0, C1), (1, C2)):
            e, off, cnt = slots[cap][core]
            xe = np.zeros((cap, D), np.float32)
            if cnt:
                xe[:cnt] = xf[toks[e][off : off + cnt]]
            m[f"xt{s}"] = np.ascontiguousarray(
                xe.T.reshape(KD, P, cap).transpose(1, 0, 2)
            ).astype(bf16)
            m[f"w1s{s}"] = w_maps[e]["w1"]
            m[f"b1s{s}"] = w_maps[e]["b1"]
            m[f"w2s{s}"] = w_maps[e]["w2"]
        in_maps.append(m)

    res = run_bass_kernel_spmd(nc, in_maps, core_ids=list(range(NCORES)))
    LAST_RESULT = res

    out = np.zeros((T, D), np.float32)
    b2f = np.asarray(b2, dtype=np.float32)
    for core in range(NCORES):
        for s, cap in ((0, C1), (1, C2)):
            e, off, cnt = slots[cap][core]
            if not cnt:
                continue
            ye = np.asarray(res.results[core][f"ye{s}"], dtype=np.float32).reshape(
                cap, D
            )[:cnt]
            tk = toks[e][off : off + cnt]
            g = gates[e][off : off + cnt]
            out[tk] += g[:, None] * (ye + b2f[e])
    return out.reshape(B, S, D)


# revision 3
# speedup vs baseline: 1.1578x; 1.0210x over previous
"""MoE (top-2 of 8 experts, d=1024, h=4096) on 8 Trainium2 NeuronCores.

Expert-parallel with 2-slot load balancing: each core runs two weight
"segments" (C1 + C2 tokens, same shapes on every core = SPMD); the host
packs expert token sets into the 16 slots (an expert may span several
slots) so per-core work is C1+C2 ~ 2176 instead of max-expert-count 2304.

Per segment, fused on-chip FFN in bf16:
  G1: hidT[h, tok] = relu(W1.T @ x.T + b1)  -- psum f32, ACT evicts to bf16
  G2: ye[tok, d]   = hidT.T @ W2            -- hid stays in SBUF (no DRAM
                                               round-trip), W2 resident bf16
Emission keeps the PE stream gap-free (the TimelineSim p-state model
penalizes any idle with ~3us of half-rate ramp); a short warmup matmul
chain covers the initial DMA wait.

Host: fp64 gating/top-2 routing, slot packing, bf16 conversion, and the
gate-weighted combine (+b2).

Self-contained: hardcodes all shapes; only imports concourse (system lib).
"""

import os

os.environ.setdefault("JAX_PLATFORMS", "")

import numpy as np

import concourse.bacc as bacc
import concourse.mybir as mybir
import concourse.tile as tile
from concourse.bass_utils import run_bass_kernel_spmd

P = 128
D = 1024  # embed dim
H = 4096  # hidden dim
E = 8  # experts
TOPK = 2
KD = D // P  # 8  k-tiles over embed
KH = H // P  # 32 k-tiles over hidden
NCORES = 8
FD = 512  # psum bank free dim (f32)

NWARM = int(os.environ.get("MOE_NWARM", "18"))
W1BUFS = int(os.environ.get("MOE_W1B", "6"))
W1PRE = int(os.environ.get("MOE_W1PRE", "4"))

_compiled = {}
LAST_RESULT = None


def _chunks(C, first):
    """Token chunks for G1's moving dim. Widths in [256, 512] (>=256 keeps the
    DMA elem >= 512B, dodging the sub-512B descriptor penalty). Seg0 gets a
    384-token first chunk: small x0 DMA for a fast start, big enough that the
    first wave of hm groups covers the x1/x2 transfer time."""
    out = []
    off = 0
    if first and C >= 1024:
        out.append((0, 384))
        off = 384
    rem = C - off
    n = max(1, -(-rem // FD))
    while n > 1 and rem / n < 256:
        n -= 1
    base, extra = divmod(rem, n)
    for i in range(n):
        w = base + (1 if i < extra else 0)
        out.append((off, w))
        off += w
    return out


def _build(C1, C2):
    if (C1, C2) in _compiled:
        return _compiled[(C1, C2)]
    f32 = mybir.dt.float32
    bf16 = mybir.dt.bfloat16
    relu = mybir.ActivationFunctionType.Relu

    nc = bacc.Bacc(None, target_bir_lowering=False)
    segs = []
    for s, C in ((0, C1), (1, C2)):
        xt_d = nc.dram_tensor(f"xt{s}", [P, KD, C], bf16, kind="ExternalInput")
        w1_d = nc.dram_tensor(f"w1s{s}", [P, KH, KD, P], bf16, kind="ExternalInput")
        b1_d = nc.dram_tensor(f"b1s{s}", [P, KH], f32, kind="ExternalInput")
        w2_d = nc.dram_tensor(f"w2s{s}", [KH, P, D], bf16, kind="ExternalInput")
        # ye in bf16: halves the output DMA (incl. the one on the critical
        # tail) and enables the DVE 2x copy mode; ~0.1% extra output error.
        # Slot sizes need not be multiples of 128: G2 runs ceil(C/128) token
        # tiles (its row count only depends on tiles), G1 rows scale with the
        # exact C -- so fractional-tile slots save real PE time.
        ye_d = nc.dram_tensor(f"ye{s}", [-(-C // P), P, D], bf16, kind="ExternalOutput")
        segs.append((s, C, xt_d, w1_d, b1_d, w2_d, ye_d))

    with tile.TileContext(nc) as tc:
        with (
            tc.tile_pool(name="warm_p", bufs=1) as warm_p,
            tc.tile_pool(name="x_p", bufs=1) as x_p,
            tc.tile_pool(name="b1_p", bufs=1) as b1_p,
            tc.tile_pool(name="w1_p", bufs=W1BUFS) as w1_p,
            tc.tile_pool(name="hid_p", bufs=1) as hid_p,
            tc.tile_pool(name="w2_p", bufs=1) as w2_p,
            tc.tile_pool(name="ob_p", bufs=3) as ob_p,
            tc.tile_pool(name="ps1", bufs=4, space="PSUM") as ps1,
            tc.tile_pool(name="ps2", bufs=4, space="PSUM") as ps2,
        ):
            # --- PE warmup: keep the tensor engine busy (p-state ramp) while
            # the first real weight/x DMAs land.
            if NWARM:
                warm = warm_p.tile([P, 2 * P], bf16, name="warm")
                nc.vector.memset(warm[:], 0.125)
                for i in range(NWARM):
                    wp = ps1.tile([P, FD], f32, tag="ps1", name=f"wps_{i}")
                    nc.tensor.matmul(
                        wp[:, :P], warm[:, :P], warm[:, P:], start=True, stop=True
                    )

            w2ts = {}
            st = [
                {"w1t": {}, "xc": {}, "chunks": _chunks(seg[1], first=(seg[0] == 0))}
                for seg in segs
            ]

            def load_w1(s, hm, eng):
                t = w1_p.tile([P, KD, P], bf16, tag="w1", name=f"w1_{s}_{hm}")
                eng.dma_start(t[:], segs[s][3][:, hm])
                st[s]["w1t"][hm] = t

            def load_x(s, ci, eng):
                # x and b1 tags are SHARED across segments (rotating bufs):
                # seg1's loads then WAR on seg0's last reads, so they cannot
                # dispatch at t=0 and front-run seg0's critical startup
                # transfers on the shared DMA engines. Tiles are allocated at
                # the max chunk width so the shared tag's buffer fits every
                # segment's chunk shape.
                off, w = st[s]["chunks"][ci]
                t = x_p.tile([P, KD, FD], bf16, tag=f"x{ci}", name=f"x_{s}_{ci}")
                eng.dma_start(t[:, :, :w], segs[s][2][:, :, off : off + w])
                st[s]["xc"][ci] = t

            def startup_loads(s, eng):
                # Ring order == PE consumption order: w1[0], x0, b1 (needed at
                # the FIRST eviction - psum recycling stalls on it), the rest
                # of the first stationary block, then x1, x2.
                load_x(s, 0, eng)
                load_w1(s, 0, eng)
                b1s = b1_p.tile([P, KH], f32, tag="b1", name=f"b1_{s}")
                eng.dma_start(b1s[:], segs[s][4][:])
                st[s]["b1"] = b1s
                for hm in range(1, W1BUFS):
                    load_w1(s, hm, eng)
                for ci in range(1, len(st[s]["chunks"])):
                    load_x(s, ci, eng)

            def g1(s):
                chunks = st[s]["chunks"]
                w1t, xc, b1s = st[s]["w1t"], st[s]["xc"], st[s]["b1"]
                C = segs[s][1]
                # Schedule: hm-major/chunk-inner (W1 streamed through W1BUFS
                # tiles), except seg0's first block runs chunk-0-only first (a
                # wave): early PE work needs only the small x0 while x1/x2 are
                # still in flight.
                NCH = len(chunks)
                sched = []
                for b0 in range(0, KH, W1BUFS):
                    hs = list(range(b0, min(b0 + W1BUFS, KH)))
                    if b0 == 0 and s == 0:
                        sched += [(hm, 0) for hm in hs]
                        sched += [(hm, c) for hm in hs for c in range(1, NCH)]
                    else:
                        sched += [(hm, c) for hm in hs for c in range(NCH)]

                # W2 tiles stream 2 per first-touched hm from hm index 8 on
                # (ACT ring): late enough to keep early DMA for x/W1, early
                # enough to finish well before G2 needs them. For s=1 each
                # load WARs on seg0 G2's last read of that k tile.
                Cpad = -(-C // P) * P  # G2 reads whole 128-token tiles
                hid = hid_p.tile([P, KH, Cpad], bf16, tag="hid", name=f"hid_{s}")
                st[s]["hid"] = hid
                if Cpad > C:
                    # G2's last tile reads the pad columns; zero them once so
                    # the garbage rows it produces are finite (host drops them)
                    nc.vector.memset(hid[:, :, C:], 0.0)
                seen = set()
                w2_next = 0
                for hm, ci in sched:
                    if hm not in seen:
                        seen.add(hm)
                        nxt = hm + W1BUFS
                        if nxt < KH:
                            load_w1(s, nxt, nc.sync)
                        if len(seen) > 8:
                            for _ in range(2):
                                if w2_next < KH:
                                    t = w2_p.tile(
                                        [P, D], bf16, tag=f"w2_{w2_next}",
                                        name=f"w2_{s}_{w2_next}",
                                    )
                                    nc.scalar.dma_start(t[:], segs[s][5][w2_next])
                                    w2ts[w2_next] = t
                                    w2_next += 1
                    off, w = chunks[ci]
                    wt = w1t[hm]
                    pt = ps1.tile([P, FD], f32, tag="ps1", name=f"p1_{s}_{hm}_{ci}")
                    for k in range(KD):
                        nc.tensor.matmul(
                            pt[:, :w],
                            wt[:, k, :],
                            xc[ci][:, k, :w],
                            start=(k == 0),
                            stop=(k == KD - 1),
                        )
                    nc.scalar.activation(
                        hid[:, hm, off : off + w],
                        pt[:, :w],
                        relu,
                        bias=b1s[:, hm : hm + 1],
                    )

            def g2(s):
                C, ye_d = segs[s][1], segs[s][6]
                hid = st[s]["hid"]
                TM = -(-C // P)
                for tm in range(TM):
                    ob = ob_p.tile([P, D], bf16, tag="ob", name=f"ob_{s}_{tm}")
                    for n in range(D // FD):
                        pt2 = ps2.tile([P, FD], f32, tag="ps2", name=f"p2_{s}_{tm}_{n}")
                        for k in range(KH):
                            nc.tensor.matmul(
                                pt2[:],
                                hid[:, k, tm * P : (tm + 1) * P],
                                w2ts[k][:, n * FD : (n + 1) * FD],
                                start=(k == 0),
                                stop=(k == KH - 1),
                            )
                        nc.vector.tensor_copy(ob[:, n * FD : (n + 1) * FD], pt2[:])
                        # ye rides the ACT ring, except the very last transfer
                        # which takes the (empty) SP ring so the final two
                        # output DMAs overlap instead of serializing.
                        last = s == 1 and tm == TM - 1 and n == 1
                        (nc.sync if last else nc.scalar).dma_start(
                            ye_d[tm][:, n * FD : (n + 1) * FD],
                            ob[:, n * FD : (n + 1) * FD],
                        )

            # Emission order: seg0 startup (ACT ring, exact FIFO control),
            # seg0 G1, then seg1 startup on the SP ring -- emitted HERE so its
            # queue position is behind seg0's WAR-gated w1 stream (otherwise
            # its dependency-free x loads would front-run seg0's critical
            # startup transfers on the shared DMA engines).
            startup_loads(0, nc.scalar)
            g1(0)
            startup_loads(1, nc.sync)
            g2(0)
            g1(1)
            g2(1)

    nc.compile()
    _compiled[(C1, C2)] = nc
    return nc


# ---------------- host side ----------------


def _try_pack(counts, C1, C2):
    """Assign each expert (a,b) = (#C1-slots, #C2-slots) covering its count,
    within 8 slots of each size. DFS biggest-expert-first, min-waste combos."""
    order = sorted(range(len(counts)), key=lambda e: -counts[e])
    assign = {}

    def dfs(i, n1, n2):
        if i == len(order):
            return True
        c = counts[order[i]]
        combos = [
            (a, b)
            for a in range(n1 + 1)
            for b in range(n2 + 1)
            if a * C1 + b * C2 >= c
        ]
        combos.sort(key=lambda ab: (ab[0] * C1 + ab[1] * C2, ab[0] + ab[1]))
        for a, b in combos[:6]:
            assign[order[i]] = (a, b)
            if dfs(i + 1, n1 - a, n2 - b):
                return True
        assign.pop(order[i], None)
        return False

    return dict(assign) if dfs(0, NCORES, NCORES) else None


def _solve_packing(counts):
    """Slot sizes (C1, C2) minimizing per-core PE rows:
    256*(C1+C2) for G1 plus 32768*(ceil(C1/128)+ceil(C2/128)) for G2.
    Sizes are arbitrary integers (G2 pads the last tile); C1 capped so
    hid+W2+x fit in SBUF."""
    total = sum(counts)
    best = None
    lo = max(256, -(-total // NCORES))
    for T in range(lo, lo + 416, 4):
        if best and 256 * T + 32768 * 17 >= best[0]:
            break
        for C1 in range(-(-T // 2), min(T - 256, 1280) + 1, 2):
            C2 = T - C1
            cost = 256 * T + 32768 * (-(-C1 // P) + (-(-C2 // P)))
            if best and cost >= best[0]:
                continue
            a = _try_pack(counts, C1, C2)
            if a is not None:
                best = (cost, C1, C2, a)
    if best is None:
        raise RuntimeError(f"no packing for counts={counts}")
    return best[1], best[2], best[3]


def kernel(x, Wg, bg, W1, b1, W2, b2):
    global LAST_RESULT
    import ml_dtypes

    bf16 = ml_dtypes.bfloat16
    x = np.ascontiguousarray(x, dtype=np.float32)
    B, S, d = x.shape
    assert d == D
    T = B * S
    xf = x.reshape(T, d)

    # ---- Host gating/routing (fp64) ----
    logits = xf.astype(np.float64) @ Wg.astype(np.float64) + bg.astype(np.float64)
    mx = logits.max(axis=1, keepdims=True)
    ex = np.exp(logits - mx)
    probs = ex / ex.sum(axis=1, keepdims=True)
    top = np.argsort(-logits, axis=1, kind="stable")[:, :TOPK]  # ties -> lower idx
    gsel = np.take_along_axis(probs, top, axis=1).astype(np.float32)

    toks, gates = [], []
    for e in range(E):
        pos = top == e
        sel = pos.any(axis=1)
        toks.append(np.nonzero(sel)[0])
        gates.append((gsel * pos).sum(axis=1)[sel].astype(np.float32))
    counts = [len(t) for t in toks]

    C1, C2, assign = _solve_packing(counts)

    # Instantiate slots: per size, a list of (expert, tok_start, n_tokens).
    slots = [[], []]  # [C1 slots, C2 slots]
    for e in range(E):
        a, b = assign[e]
        rem, off = counts[e], 0
        for si, (cap, cnt) in enumerate(((C1, a), (C2, b))):
            for _ in range(cnt):
                take = min(rem, cap)
                slots[si].append((e, off, take))
                off += take
                rem -= take
    for si in range(2):
        while len(slots[si]) < NCORES:
            slots[si].append((0, 0, 0))  # unused slot: expert-0 weights, 0 toks

    # Per-expert device weight layouts (bf16), built once.
    w_maps = {}
    for e in set(sl[0] for si in range(2) for sl in slots[si]):
        w_maps[e] = {
            "w1": np.ascontiguousarray(
                np.asarray(W1[e], dtype=np.float32)
                .reshape(KD, P, KH, P)
                .transpose(1, 2, 0, 3)
            ).astype(bf16),
            "b1": np.ascontiguousarray(
                np.asarray(b1[e], dtype=np.float32).reshape(KH, P).T
            ),
            "w2": np.ascontiguousarray(W2[e], dtype=np.float32)
            .reshape(KH, P, D)
            .astype(bf16),
        }

    nc = _build(C1, C2)
    in_maps = []
    for core in range(NCORES):
        m = {}
        for s, cap in ((0, C1), (1, C2)):
            e, off, cnt = slots[s][core]
            xe = np.zeros((cap, D), np.float32)
            if cnt:
                xe[:cnt] = xf[toks[e][off : off + cnt]]
            m[f"xt{s}"] = np.ascontiguousarray(
                xe.T.reshape(KD, P, cap).transpose(1, 0, 2)
            ).astype(bf16)
            m[f"w1s{s}"] = w_maps[e]["w1"]
            m[f"b1s{s}"] = w_maps[e]["b1"]
            m[f"w2s{s}"] = w_maps[e]["w2"]
        in_maps.append(m)

    res = run_bass_kernel_spmd(nc, in_maps, core_ids=list(range(NCORES)))
    LAST_RESULT = res

    out = np.zeros((T, D), np.float32)
    b2f = np.asarray(b2, dtype=np.float32)
    for core in range(NCORES):
        for s, cap in ((0, C1), (1, C2)):
            e, off, cnt = slots[s][core]
            if not cnt:
                continue
            ye = np.asarray(res.results[core][f"ye{s}"], dtype=np.float32).reshape(
                -1, D
            )[:cnt]
            tk = toks[e][off : off + cnt]
            g = gates[e][off : off + cnt]
            out[tk] += g[:, None] * (ye + b2f[e])
    return out.reshape(B, S, D)


# revision 4
# speedup vs baseline: 1.1578x; 1.0001x over previous
"""MoE (top-2 of 8 experts, d=1024, h=4096) on 8 Trainium2 NeuronCores.

Expert-parallel with 2-slot load balancing: each core runs two weight
"segments" (C1 + C2 tokens, same shapes on every core = SPMD); the host
packs expert token sets into the 16 slots (an expert may span several
slots) so per-core work is C1+C2 ~ 2176 instead of max-expert-count 2304.

Per segment, fused on-chip FFN in bf16:
  G1: hidT[h, tok] = relu(W1.T @ x.T + b1)  -- psum f32, ACT evicts to bf16
  G2: ye[tok, d]   = hidT.T @ W2            -- hid stays in SBUF (no DRAM
                                               round-trip), W2 resident bf16
Emission keeps the PE stream gap-free (the TimelineSim p-state model
penalizes any idle with ~3us of half-rate ramp); a short warmup matmul
chain covers the initial DMA wait.

Host: fp64 gating/top-2 routing, slot packing, bf16 conversion, and the
gate-weighted combine (+b2).

Self-contained: hardcodes all shapes; only imports concourse (system lib).
"""

import os

os.environ.setdefault("JAX_PLATFORMS", "")

import numpy as np

import concourse.bacc as bacc
import concourse.mybir as mybir
import concourse.tile as tile
from concourse.bass_utils import run_bass_kernel_spmd

P = 128
D = 1024  # embed dim
H = 4096  # hidden dim
E = 8  # experts
TOPK = 2
KD = D // P  # 8  k-tiles over embed
KH = H // P  # 32 k-tiles over hidden
NCORES = 8
FD = 512  # psum bank free dim (f32)

NWARM = int(os.environ.get("MOE_NWARM", "18"))
W1BUFS = int(os.environ.get("MOE_W1B", "6"))
W1PRE = int(os.environ.get("MOE_W1PRE", "4"))

_compiled = {}
LAST_RESULT = None


def _chunks(C, first):
    """Token chunks for G1's moving dim. Widths in [256, 512] (>=256 keeps the
    DMA elem >= 512B, dodging the sub-512B descriptor penalty). Seg0 gets a
    384-token first chunk: small x0 DMA for a fast start, big enough that the
    first wave of hm groups covers the x1/x2 transfer time."""
    out = []
    off = 0
    if first and C >= 1024:
        out.append((0, 320))
        off = 320
    rem = C - off
    n = max(1, -(-rem // FD))
    while n > 1 and rem / n < 256:
        n -= 1
    base, extra = divmod(rem, n)
    for i in range(n):
        w = base + (1 if i < extra else 0)
        out.append((off, w))
        off += w
    return out


def _build(C1, C2):
    if (C1, C2) in _compiled:
        return _compiled[(C1, C2)]
    f32 = mybir.dt.float32
    bf16 = mybir.dt.bfloat16
    relu = mybir.ActivationFunctionType.Relu

    nc = bacc.Bacc(None, target_bir_lowering=False)
    segs = []
    for s, C in ((0, C1), (1, C2)):
        xt_d = nc.dram_tensor(f"xt{s}", [P, KD, C], bf16, kind="ExternalInput")
        w1_d = nc.dram_tensor(f"w1s{s}", [P, KH, KD, P], bf16, kind="ExternalInput")
        b1_d = nc.dram_tensor(f"b1s{s}", [P, KH], f32, kind="ExternalInput")
        w2_d = nc.dram_tensor(f"w2s{s}", [KH, P, D], bf16, kind="ExternalInput")
        # ye in bf16: halves the output DMA (incl. the one on the critical
        # tail) and enables the DVE 2x copy mode; ~0.1% extra output error.
        # Slot sizes need not be multiples of 128: G2 runs ceil(C/128) token
        # tiles (its row count only depends on tiles), G1 rows scale with the
        # exact C -- so fractional-tile slots save real PE time.
        ye_d = nc.dram_tensor(f"ye{s}", [-(-C // P), P, D], bf16, kind="ExternalOutput")
        segs.append((s, C, xt_d, w1_d, b1_d, w2_d, ye_d))

    with tile.TileContext(nc) as tc:
        with (
            tc.tile_pool(name="warm_p", bufs=1) as warm_p,
            tc.tile_pool(name="x_p", bufs=1) as x_p,
            tc.tile_pool(name="b1_p", bufs=1) as b1_p,
            tc.tile_pool(name="w1_p", bufs=W1BUFS) as w1_p,
            tc.tile_pool(name="hid_p", bufs=1) as hid_p,
            tc.tile_pool(name="w2_p", bufs=1) as w2_p,
            tc.tile_pool(name="ob_p", bufs=3) as ob_p,
            tc.tile_pool(name="ps1", bufs=4, space="PSUM") as ps1,
            tc.tile_pool(name="ps2", bufs=4, space="PSUM") as ps2,
        ):
            # --- PE warmup: keep the tensor engine busy (p-state ramp) while
            # the first real weight/x DMAs land.
            if NWARM:
                warm = warm_p.tile([P, 2 * P], bf16, name="warm")
                nc.vector.memset(warm[:], 0.125)
                for i in range(NWARM):
                    wp = ps1.tile([P, FD], f32, tag="ps1", name=f"wps_{i}")
                    nc.tensor.matmul(
                        wp[:, :P], warm[:, :P], warm[:, P:], start=True, stop=True
                    )

            w2ts = {}
            st = [
                {"w1t": {}, "xc": {}, "chunks": _chunks(seg[1], first=(seg[0] == 0))}
                for seg in segs
            ]

            def load_w1(s, hm, eng):
                t = w1_p.tile([P, KD, P], bf16, tag="w1", name=f"w1_{s}_{hm}")
                eng.dma_start(t[:], segs[s][3][:, hm])
                st[s]["w1t"][hm] = t

            def load_x(s, ci, eng):
                # x and b1 tags are SHARED across segments (rotating bufs):
                # seg1's loads then WAR on seg0's last reads, so they cannot
                # dispatch at t=0 and front-run seg0's critical startup
                # transfers on the shared DMA engines. Tiles are allocated at
                # the max chunk width so the shared tag's buffer fits every
                # segment's chunk shape.
                off, w = st[s]["chunks"][ci]
                t = x_p.tile([P, KD, FD], bf16, tag=f"x{ci}", name=f"x_{s}_{ci}")
                eng.dma_start(t[:, :, :w], segs[s][2][:, :, off : off + w])
                st[s]["xc"][ci] = t

            def startup_loads(s, eng):
                # Ring order == PE consumption order: w1[0], x0, b1 (needed at
                # the FIRST eviction - psum recycling stalls on it), the rest
                # of the first stationary block, then x1, x2.
                load_x(s, 0, eng)
                load_w1(s, 0, eng)
                b1s = b1_p.tile([P, KH], f32, tag="b1", name=f"b1_{s}")
                eng.dma_start(b1s[:], segs[s][4][:])
                st[s]["b1"] = b1s
                for hm in range(1, W1BUFS):
                    load_w1(s, hm, eng)
                for ci in range(1, len(st[s]["chunks"])):
                    load_x(s, ci, eng)

            def g1(s):
                chunks = st[s]["chunks"]
                w1t, xc, b1s = st[s]["w1t"], st[s]["xc"], st[s]["b1"]
                C = segs[s][1]
                # Schedule: hm-major/chunk-inner (W1 streamed through W1BUFS
                # tiles), except seg0's first block runs chunk-0-only first (a
                # wave): early PE work needs only the small x0 while x1/x2 are
                # still in flight.
                NCH = len(chunks)
                sched = []
                for b0 in range(0, KH, W1BUFS):
                    hs = list(range(b0, min(b0 + W1BUFS, KH)))
                    if b0 == 0 and s == 0:
                        sched += [(hm, 0) for hm in hs]
                        sched += [(hm, c) for hm in hs for c in range(1, NCH)]
                    else:
                        sched += [(hm, c) for hm in hs for c in range(NCH)]

                # W2 tiles stream 2 per first-touched hm from hm index 8 on
                # (ACT ring): late enough to keep early DMA for x/W1, early
                # enough to finish well before G2 needs them. For s=1 each
                # load WARs on seg0 G2's last read of that k tile.
                Cpad = -(-C // P) * P  # G2 reads whole 128-token tiles
                hid = hid_p.tile([P, KH, Cpad], bf16, tag="hid", name=f"hid_{s}")
                st[s]["hid"] = hid
                if Cpad > C:
                    # G2's last tile reads the pad columns; zero them once so
                    # the garbage rows it produces are finite (host drops them)
                    nc.vector.memset(hid[:, :, C:], 0.0)
                seen = set()
                w2_next = 0
                for hm, ci in sched:
                    if hm not in seen:
                        seen.add(hm)
                        nxt = hm + W1BUFS
                        if nxt < KH:
                            load_w1(s, nxt, nc.sync)
                        if len(seen) > 8:
                            for _ in range(2):
                                if w2_next < KH:
                                    t = w2_p.tile(
                                        [P, D], bf16, tag=f"w2_{w2_next}",
                                        name=f"w2_{s}_{w2_next}",
                                    )
                                    nc.scalar.dma_start(t[:], segs[s][5][w2_next])
                                    w2ts[w2_next] = t
                                    w2_next += 1
                    off, w = chunks[ci]
                    wt = w1t[hm]
                    pt = ps1.tile([P, FD], f32, tag="ps1", name=f"p1_{s}_{hm}_{ci}")
                    for k in range(KD):
                        nc.tensor.matmul(
                            pt[:, :w],
                            wt[:, k, :],
                            xc[ci][:, k, :w],
                            start=(k == 0),
                            stop=(k == KD - 1),
                        )
                    nc.scalar.activation(
                        hid[:, hm, off : off + w],
                        pt[:, :w],
                        relu,
                        bias=b1s[:, hm : hm + 1],
                    )

            def g2(s):
                C, ye_d = segs[s][1], segs[s][6]
                hid = st[s]["hid"]
                TM = -(-C // P)
                for tm in range(TM):
                    ob = ob_p.tile([P, D], bf16, tag="ob", name=f"ob_{s}_{tm}")
                    for n in range(D // FD):
                        pt2 = ps2.tile([P, FD], f32, tag="ps2", name=f"p2_{s}_{tm}_{n}")
                        for k in range(KH):
                            nc.tensor.matmul(
                                pt2[:],
                                hid[:, k, tm * P : (tm + 1) * P],
                                w2ts[k][:, n * FD : (n + 1) * FD],
                                start=(k == 0),
                                stop=(k == KH - 1),
                            )
                        nc.vector.tensor_copy(ob[:, n * FD : (n + 1) * FD], pt2[:])
                        # ye rides the ACT ring, except the very last transfer
                        # which takes the (empty) SP ring so the final two
                        # output DMAs overlap instead of serializing.
                        last = s == 1 and tm == TM - 1 and n == 1
                        (nc.sync if last else nc.scalar).dma_start(
                            ye_d[tm][:, n * FD : (n + 1) * FD],
                            ob[:, n * FD : (n + 1) * FD],
                        )

            # Emission order: seg0 startup (ACT ring, exact FIFO control),
            # seg0 G1, then seg1 startup on the SP ring -- emitted HERE so its
            # queue position is behind seg0's WAR-gated w1 stream (otherwise
            # its dependency-free x loads would front-run seg0's critical
            # startup transfers on the shared DMA engines).
            startup_loads(0, nc.scalar)
            g1(0)
            startup_loads(1, nc.sync)
            g2(0)
            g1(1)
            g2(1)

    nc.compile()
    _compiled[(C1, C2)] = nc
    return nc


# ---------------- host side ----------------


def _try_pack(counts, C1, C2):
    """Assign each expert (a,b) = (#C1-slots, #C2-slots) covering its count,
    within 8 slots of each size. DFS biggest-expert-first, min-waste combos."""
    order = sorted(range(len(counts)), key=lambda e: -counts[e])
    assign = {}

    def dfs(i, n1, n2):
        if i == len(order):
            return True
        c = counts[order[i]]
        combos = [
            (a, b)
            for a in range(n1 + 1)
            for b in range(n2 + 1)
            if a * C1 + b * C2 >= c
        ]
        combos.sort(key=lambda ab: (ab[0] * C1 + ab[1] * C2, ab[0] + ab[1]))
        for a, b in combos[:6]:
            assign[order[i]] = (a, b)
            if dfs(i + 1, n1 - a, n2 - b):
                return True
        assign.pop(order[i], None)
        return False

    return dict(assign) if dfs(0, NCORES, NCORES) else None


def _solve_packing(counts):
    """Slot sizes (C1, C2) minimizing per-core PE rows:
    256*(C1+C2) for G1 plus 32768*(ceil(C1/128)+ceil(C2/128)) for G2.
    Sizes are arbitrary integers (G2 pads the last tile); C1 capped so
    hid+W2+x fit in SBUF."""
    total = sum(counts)
    best = None
    lo = max(256, -(-total // NCORES))
    mintile = -(-lo // P)
    # C1 >= ceil(max_count/2): the largest expert fits in two C1 slots (using
    # 3+ slots for it is strictly more fragmented). Fallback widens the scan.
    c1lo = -(-max(counts) // 2)
    for widen in (False, True):
        for T in range(lo, lo + 416, 4):
            if best and 256 * T + 32768 * mintile >= best[0]:
                break
            start = -(-T // 2) if widen else max(-(-T // 2), c1lo)
            for C1 in range(start, min(T - 256, 1280) + 1, 2):
                C2 = T - C1
                cost = 256 * T + 32768 * (-(-C1 // P) + (-(-C2 // P)))
                if best and cost >= best[0]:
                    continue
                # cheap necessary condition: per-expert min waste (ignoring
                # slot exhaustion) must fit in the total slack
                slack = NCORES * T - total
                mw = 0
                for c in counts:
                    best_alloc = min(
                        (a * C1 + b * C2 for a in range(3) for b in range(3)
                         if a * C1 + b * C2 >= c),
                        default=None,
                    )
                    if best_alloc is None:
                        mw = slack + 1
                        break
                    mw += best_alloc - c
                if mw > slack:
                    continue
                a = _try_pack(counts, C1, C2)
                if a is not None:
                    best = (cost, C1, C2, a)
        if best is not None:
            break
    if best is None:
        raise RuntimeError(f"no packing for counts={counts}")
    return best[1], best[2], best[3]


def kernel(x, Wg, bg, W1, b1, W2, b2):
    global LAST_RESULT
    import ml_dtypes

    bf16 = ml_dtypes.bfloat16
    x = np.ascontiguousarray(x, dtype=np.float32)
    B, S, d = x.shape
    assert d == D
    T = B * S
    xf = x.reshape(T, d)

    # ---- Host gating/routing (fp64) ----
    logits = xf.astype(np.float64) @ Wg.astype(np.float64) + bg.astype(np.float64)
    mx = logits.max(axis=1, keepdims=True)
    ex = np.exp(logits - mx)
    probs = ex / ex.sum(axis=1, keepdims=True)
    top = np.argsort(-logits, axis=1, kind="stable")[:, :TOPK]  # ties -> lower idx
    gsel = np.take_along_axis(probs, top, axis=1).astype(np.float32)

    toks, gates = [], []
    for e in range(E):
        pos = top == e
        sel = pos.any(axis=1)
        toks.append(np.nonzero(sel)[0])
        gates.append((gsel * pos).sum(axis=1)[sel].astype(np.float32))
    counts = [len(t) for t in toks]

    C1, C2, assign = _solve_packing(counts)

    # Instantiate slots: per size, a list of (expert, tok_start, n_tokens).
    slots = [[], []]  # [C1 slots, C2 slots]
    for e in range(E):
        a, b = assign[e]
        rem, off = counts[e], 0
        for si, (cap, cnt) in enumerate(((C1, a), (C2, b))):
            for _ in range(cnt):
                take = min(rem, cap)
                slots[si].append((e, off, take))
                off += take
                rem -= take
    for si in range(2):
        while len(slots[si]) < NCORES:
            slots[si].append((0, 0, 0))  # unused slot: expert-0 weights, 0 toks

    # Per-expert device weight layouts (bf16), built once.
    w_maps = {}
    for e in set(sl[0] for si in range(2) for sl in slots[si]):
        w_maps[e] = {
            "w1": np.ascontiguousarray(
                np.asarray(W1[e], dtype=np.float32)
                .reshape(KD, P, KH, P)
                .transpose(1, 2, 0, 3)
            ).astype(bf16),
            "b1": np.ascontiguousarray(
                np.asarray(b1[e], dtype=np.float32).reshape(KH, P).T
            ),
            "w2": np.ascontiguousarray(W2[e], dtype=np.float32)
            .reshape(KH, P, D)
            .astype(bf16),
        }

    nc = _build(C1, C2)
    in_maps = []
    for core in range(NCORES):
        m = {}
        for s, cap in ((0, C1), (1, C2)):
            e, off, cnt = slots[s][core]
            xe = np.zeros((cap, D), np.float32)
            if cnt:
                xe[:cnt] = xf[toks[e][off : off + cnt]]
            m[f"xt{s}"] = np.ascontiguousarray(
                xe.T.reshape(KD, P, cap).transpose(1, 0, 2)
            ).astype(bf16)
            m[f"w1s{s}"] = w_maps[e]["w1"]
            m[f"b1s{s}"] = w_maps[e]["b1"]
            m[f"w2s{s}"] = w_maps[e]["w2"]
        in_maps.append(m)

    res = run_bass_kernel_spmd(nc, in_maps, core_ids=list(range(NCORES)))
    LAST_RESULT = res

    out = np.zeros((T, D), np.float32)
    b2f = np.asarray(b2, dtype=np.float32)
    for core in range(NCORES):
        for s, cap in ((0, C1), (1, C2)):
            e, off, cnt = slots[s][core]
            if not cnt:
                continue
            ye = np.asarray(res.results[core][f"ye{s}"], dtype=np.float32).reshape(
                -1, D
            )[:cnt]
            tk = toks[e][off : off + cnt]
            g = gates[e][off : off + cnt]
            out[tk] += g[:, None] * (ye + b2f[e])
    return out.reshape(B, S, D)
